# revision 6
# baseline (speedup 1.0000x reference)
"""LoRA layer kernel for Trainium2 (Bass/Tile), data-parallel over 8 NeuronCores.

Math:  out = (x @ B) @ A * (32/16)   with x [4,2048,4096], B [4096,16], A [16,4096].

Strategy (v3 — overlap-focused):
  - Flatten tokens (4*2048=8192), shard 1024 tokens per core (data parallel).
  - x pre-tiled PARTITION-MAJOR on host as [ntb, 128, NB, tb] f16 so each
    block load is 128 fully-contiguous 16 KB descriptors.
  - Load order: x block 0 first (it gates mm1), then the tiny consts, then
    the remaining x blocks — all on the sync queue so block 0 drains first.
  - Stores are dispatched from the SYNC engine (idle after loads), so they
    are never queued behind ACT's copy work.
  - mm1: 4-way column-group packed fp16 matmuls (chunk 4k+g accumulates
    into PSUM partitions [32g, 32g+16)); selector matmul folds the 4
    partial groups into one [16, t] xbT.
  - mm2: fp16, row-group packed over token subtiles; weights stay resident
    (one explicit ldweights per subtile per block, matmuls with
    ldweights=False). Output copies: subtile 0 -> DVE, subtile 1 -> ACT so
    each store gates on exactly one engine; 2-bank PSUM tiles halve the
    per-copy fixed cost.
"""

import os
import numpy as np

IN = 4096
OUT = 4096
R = 16
N_CORES = 8
SCALE = 32.0 / 16.0
P = 128
NB = IN // P  # 32 contraction chunks


def _install_profile_hook():
    """Best-effort: register the axon NTFF profiling hook that this image's
    `antenv` package is missing, so run_bass_kernel_spmd(trace=True) can
    return exec_time_ns. Harmless no-op when anything is unavailable."""
    try:
        import sys
        import types

        if "antenv.axon_hooks" in sys.modules:
            return
        try:
            import antenv  # noqa: F401
        except ImportError:
            return
        mod = types.ModuleType("antenv.axon_hooks")
        mod._hook = None

        def set_axon_ntff_profile_hook(h):
            mod._hook = h

        def get_axon_ntff_profile_hook():
            return mod._hook

        mod.set_axon_ntff_profile_hook = set_axon_ntff_profile_hook
        mod.get_axon_ntff_profile_hook = get_axon_ntff_profile_hook
        sys.modules["antenv.axon_hooks"] = mod
        import antenv as _antenv

        _antenv.axon_hooks = mod

        so_path = "/opt/axon/libaxon_pjrt.so"
        if os.path.exists(so_path):
            try:
                from trn_agent_boot.trn_boot import _ntff_profile_via_ctypes

                hook = _ntff_profile_via_ctypes(so_path)
                if hook is not None:
                    mod._hook = hook
            except Exception:
                pass
    except Exception:
        pass


_install_profile_hook()

_NC_CACHE = {}


def build_nc(tok, tb=256):
    """Build + compile the per-core Bass program for `tok` tokens/core."""
    key = (tok, tb)
    if key in _NC_CACHE:
        return _NC_CACHE[key]

    import concourse.bacc as bacc
    import concourse.tile as tile
    from concourse import mybir

    f32 = mybir.dt.float32
    f16 = mybir.dt.float16
    tb = min(tb, tok)
    assert tok % tb == 0 and tb % P == 0
    ntb = tok // tb
    nst = tb // P  # token subtiles per block

    nc = bacc.Bacc("TRN2", target_bir_lowering=False, debug=False)
    xT = nc.dram_tensor("xT", [ntb, P, NB, tb], f16, kind="ExternalInput").ap()
    Bt = nc.dram_tensor("Bt", [P, NB, 2 * R], f16, kind="ExternalInput").ap()
    Af = nc.dram_tensor("Af", [R, OUT], f16, kind="ExternalInput").ap()
    Ss = nc.dram_tensor("Ss", [P, R], f16, kind="ExternalInput").ap()
    out = nc.dram_tensor("out", [tok, OUT], f16, kind="ExternalOutput").ap()

    with tile.TileContext(nc) as tc:
        with (
            tc.tile_pool(name="const", bufs=1) as const_pool,
            tc.tile_pool(name="xin", bufs=min(4, ntb)) as x_pool,
            tc.tile_pool(name="xbt", bufs=2) as xbt_pool,
            tc.tile_pool(name="ps1", bufs=2, space="PSUM") as ps1,
            tc.tile_pool(name="psS", bufs=1, space="PSUM") as psS,
            tc.tile_pool(name="ps2", bufs=1, space="PSUM") as ps2,
            tc.tile_pool(name="osb", bufs=4) as out_pool,
        ):
            # x block 0 load goes first — it gates the whole compute chain
            xT_sbs = [
                x_pool.tile([P, NB, tb], f16, name=f"x{i}", tag="x")
                for i in range(ntb)
            ]
            nc.sync.dma_start(out=xT_sbs[0][:], in_=xT[0])

            B_sb = const_pool.tile([P, NB, 2 * R], f16)
            nc.sync.dma_start(out=B_sb[:], in_=Bt[:])
            # A loaded compact, replicated on-chip to rows 32g+r
            A_sb = const_pool.tile([P, OUT], f16)
            nc.sync.dma_start(out=A_sb[:R, :], in_=Af[:])
            for g in range(1, 4):
                nc.vector.tensor_copy(A_sb[32 * g : 32 * g + R, :], A_sb[:R, :])
            # selector: S[32g+r, r] = 1 -> matmul with S sums the 4 col-group
            # partials back into a single [16, t] xbT
            S_sb = const_pool.tile([P, R], f16)
            nc.sync.dma_start(out=S_sb[:], in_=Ss[:])

            for tbi in range(1, ntb):
                nc.sync.dma_start(out=xT_sbs[tbi][:], in_=xT[tbi])

            for tbi in range(ntb):
                xT_sb = xT_sbs[tbi]
                # mm1, 4-way column-group packed
                ps_part = ps1.tile([P, tb], f32)
                for c8 in range(NB // 4):
                    for g in range(4):
                        c = c8 * 4 + g
                        nc.tensor.matmul(
                            ps_part[32 * g : 32 * g + 2 * R, :],
                            lhsT=B_sb[:, c, :],
                            rhs=xT_sb[:, c, :],
                            start=(c8 == 0),
                            stop=(c8 == NB // 4 - 1),
                            tile_position=(0, 32 * g),
                            skip_group_check=True,
                        )
                part_sb = xbt_pool.tile([P, tb], f16, tag="part")
                nc.vector.tensor_copy(part_sb[:], ps_part[:])
                # selector matmul: fold the 4 col-group partials to [16, t]
                ps_xbt = psS.tile([R, tb], f32)
                nc.tensor.matmul(
                    ps_xbt[:],
                    lhsT=S_sb[:],
                    rhs=part_sb[:],
                    start=True,
                    stop=True,
                    skip_group_check=True,
                )
                # partition-shifting copies: subtile st's xbT to row group
                # 32st so the packed mm2's row-tiled matmuls run concurrently
                xbt_sb = xbt_pool.tile([P, P], f16, tag="xbt")
                for st in range(nst):
                    nc.vector.tensor_copy(
                        xbt_sb[32 * st : 32 * st + R, :],
                        ps_xbt[:, st * P : (st + 1) * P],
                    )

                # mm2: load each subtile's weights once; the matmuls then run
                # with ldweights=False (weights stay resident in disjoint
                # row strips of the PE array)
                for st in range(nst):
                    nc.tensor.ldweights(
                        xbt_sb[32 * st : 32 * st + R, :],
                        tile_position=(32 * st, 0),
                    )
                o_sbs = [
                    out_pool.tile([P, OUT], f16, name=f"osb{st}_{tbi}", tag=f"osb{st}")
                    for st in range(nst)
                ]
                for op in range(OUT // 1024):
                    for st in range(nst):
                        # 2-bank PSUM tile: two matmuls, one wide copy
                        ps_o = ps2.tile([P, 1024], f32, tag=f"ps2_{st}")
                        for h in range(2):
                            mm = nc.tensor.matmul(
                                ps_o[:, h * 512 : (h + 1) * 512],
                                lhsT=xbt_sb[32 * st : 32 * st + R, :],
                                rhs=A_sb[
                                    32 * st : 32 * st + R,
                                    (2 * op + h) * 512 : (2 * op + h + 1) * 512,
                                ],
                                start=True,
                                stop=True,
                                tile_position=(32 * st, 0),
                                skip_group_check=True,
                            )
                            mm.ldweights = False
                        # subtile 0 -> DVE, subtile 1 -> ACT: each store
                        # gates on exactly one copy engine
                        dst = o_sbs[st][:, op * 1024 : (op + 1) * 1024]
                        if st % 2 == 0:
                            nc.vector.tensor_copy(dst, ps_o[:])
                        else:
                            nc.scalar.activation(
                                dst, ps_o[:], mybir.ActivationFunctionType.Copy
                            )
                # stores dispatched from the (idle) sync engine
                for st in range(nst):
                    t0 = tbi * tb + st * P
                    nc.sync.dma_start(out=out[t0 : t0 + P, :], in_=o_sbs[st][:])

    nc.compile()
    _NC_CACHE[key] = nc
    return nc


TB = 256


def make_in_maps(x, lora_A, lora_B, n_cores=N_CORES):
    x = np.asarray(x, dtype=np.float32)
    A = np.asarray(lora_A, dtype=np.float32)
    B = np.asarray(lora_B, dtype=np.float32)
    xf = x.reshape(-1, IN)
    ntok = xf.shape[0] // n_cores
    tb = min(TB, ntok)
    A_scaled = (A * np.float32(SCALE)).astype(np.float16)
    S_sel = np.zeros((P, R), dtype=np.float16)
    for g in range(4):
        S_sel[32 * g : 32 * g + R] = np.eye(R, dtype=np.float16)
    B_resh = np.zeros((P, NB, 2 * R), dtype=np.float16)
    B_resh[:, :, :R] = B.reshape(NB, P, R).transpose(1, 0, 2)
    in_maps = []
    for c in range(n_cores):
        shard = xf[c * ntok : (c + 1) * ntok]
        # pre-tile partition-major: [ntb, 128, NB, tb];
        # xt[tbi, p, c, t] = shard[tbi*tb + t, c*128 + p]
        xt = np.ascontiguousarray(
            shard.reshape(ntok // tb, tb, NB, P).transpose(0, 3, 2, 1),
            dtype=np.float16,
        )
        in_maps.append(
            {
                "xT": xt,
                "Bt": B_resh,
                "Af": A_scaled,
                "Ss": S_sel,
            }
        )
    return in_maps, ntok


def kernel_with_results(x, lora_A, lora_B, trace=False, **kwargs):
    from concourse.bass_utils import run_bass_kernel_spmd

    in_maps, ntok = make_in_maps(x, lora_A, lora_B)
    nc = build_nc(ntok, tb=TB)
    res = run_bass_kernel_spmd(nc, in_maps, list(range(N_CORES)), trace=trace, **kwargs)
    out = np.concatenate([r["out"] for r in res.results], axis=0).astype(np.float32)
    return out.reshape(np.asarray(x).shape[:-1] + (OUT,)), res


def kernel(x, lora_A, lora_B):
    out, _ = kernel_with_results(x, lora_A, lora_B)
    return out
